# revision 7
# baseline (speedup 1.0000x reference)
"""Grouped linear (MoE expert GEMM) for Trainium2, 8-core expert-parallel.

Problem: x [16384, 1024] f32, W [64, 4096, 1024] f32, b [64, 4096] f32,
m_splits [64] int64 (host-side counts; 256 each in the reference setup).
y[t] = x[t] @ W[e].T + b[e] for tokens t owned by expert e.

Sharding: expert-parallel — core c owns experts [8c, 8c+8).

Strategy vs the fp32r-split v1 baseline (785us): the correctness gate is
rel_err < 2e-2, so cast x and W to bf16 ON THE HOST. PE runs bf16 at the
same per-row rate as fp32r but needs ONE matmul term instead of the
3-term hi/lo split -> 3x less PE work. Halved W DMA (67MB/core) and bf16
y-out (16.8MB/core) with fp32 PSUM accumulation give rel_err ~3e-3 (6x
under the gate). Measured v2: 174us.

v3 refinements (DGE issue-cost bound, not bandwidth bound):
- Host packs W/x so each partition's block data is one contiguous line
  (16-32KB): 8x fewer DMA descriptors per transfer.
- DMA issue spread across engine queues: W on SP (sync), y on Activation
  (scalar), x + bias on GPSIMD — each hwdge config costs ~0.6-1us of the
  issuing engine's sequencer, so one queue for everything serializes.
- One bias DMA per rep ([8,4096] bf16) + per-expert GPSIMD
  partition_broadcast; one merged y DMA per expert ([256,4096]).
- Matmul loop order it-outer/ob-inner so the stationary x tile is reused
  by consecutive matmuls into different PSUM banks.

Bias is added during PSUM evacuation by DVE (tensor_add with bf16 round).
"""

import numpy as np

NUM_GEMMS = 64
IN_FEATURES = 1024
OUT_FEATURES = 4096
TPE = 256  # tokens per expert slot (padded to this)
N_CORES = 8
EPC = NUM_GEMMS // N_CORES  # experts per core
TOK_PER_CORE = EPC * TPE  # 2048
IT = IN_FEATURES // 128  # 8 contraction tiles
TT = TPE // 128  # 2 token tiles per expert

_CACHE: dict = {}


DEFAULT_CFG = dict(
    obw=1024,        # W DMA block width (outs); OUT_FEATURES % obw == 0
    w_bufs=3,
    ps_bufs=4,       # per psum tile name; 2 names x 4 bufs = 8 banks
    reorder=True,    # it-outer/ob-inner matmul order (stationary reuse)
    y_eng="act",     # engine issuing y DMAs: "act" | "sp"
    bias_once=False,  # illegal: partition_broadcast src must be partition 0
    # --- timing probes (wrong results when on; never set in kernel()) ---
    probe_half_w=False,    # reuse previous W tile for odd nb (halves W DMA)
    probe_half_mm=False,   # only 4 of 8 contraction matmuls (halves PE)
    probe_skip_evac=False, # skip DVE evac + bias + y DMA
)


def _build_nc(reps: int = 1, **cfg_over):
    import concourse.bacc as bacc
    import concourse.mybir as mybir
    import concourse.tile as tile

    cfg = {**DEFAULT_CFG, **cfg_over}
    F32 = mybir.dt.float32
    BF16 = mybir.dt.bfloat16
    OBW = cfg["obw"]
    NB = OUT_FEATURES // OBW
    OB = OBW // 512  # psum column tiles per W block
    NIT = 4 if cfg["probe_half_mm"] else IT

    nc = bacc.Bacc(
        "TRN2", target_bir_lowering=False, debug=False, num_devices=N_CORES
    )
    # host-packed layouts (see _prep_*): per-partition-contiguous lines
    xT_d = nc.dram_tensor("xT", [128, IT, TOK_PER_CORE], BF16,
                          kind="ExternalInput")
    wT_d = nc.dram_tensor("wT", [EPC, NB, 128, IT, OBW], BF16,
                          kind="ExternalInput")
    b_d = nc.dram_tensor("b", [EPC, OUT_FEATURES], BF16, kind="ExternalInput")
    y_d = nc.dram_tensor("y", [TOK_PER_CORE, OUT_FEATURES], BF16,
                         kind="ExternalOutput")

    y_dma = nc.scalar.dma_start if cfg["y_eng"] == "act" else nc.sync.dma_start

    with tile.TileContext(nc) as tc:
        with (
            tc.tile_pool(name="xres", bufs=1) as x_p,
            tc.tile_pool(name="wblk", bufs=cfg["w_bufs"]) as w_p,
            tc.tile_pool(name="bias", bufs=2) as bias_p,
            tc.tile_pool(name="bbc", bufs=2) as bbc_p,
            tc.tile_pool(name="outp", bufs=2) as out_p,
            tc.tile_pool(name="ps", bufs=cfg["ps_bufs"], space="PSUM") as ps_p,
        ):
            for _rep in range(reps):
                # ---- x: one DMA, resident for the whole rep ----
                xt = x_p.tile([128, IT * TOK_PER_CORE], BF16)
                nc.gpsimd.dma_start(
                    xt[:].rearrange("p (it t) -> p it t", t=TOK_PER_CORE),
                    xT_d.ap(),
                )
                ball = None
                if cfg["bias_once"]:
                    ball = bias_p.tile([EPC, OUT_FEATURES], BF16)
                    nc.gpsimd.dma_start(ball[:], b_d.ap())
                for e in range(EPC):
                    if cfg["bias_once"]:
                        bias_src = ball[e:e + 1, :]
                    else:
                        bias_raw = bias_p.tile([1, OUT_FEATURES], BF16)
                        nc.gpsimd.dma_start(bias_raw[:], b_d.ap()[e:e + 1, :])
                        bias_src = bias_raw[:]
                    bias_bc = bbc_p.tile([128, OUT_FEATURES], BF16)
                    nc.gpsimd.partition_broadcast(bias_bc[:], bias_src)

                    outw = out_p.tile([128, TT * OUT_FEATURES], BF16)
                    prev_wt = None
                    for nb in range(NB):
                        if cfg["probe_half_w"] and nb % 2 == 1:
                            wt = prev_wt
                        else:
                            wt = w_p.tile([128, IT * OBW], BF16)
                            nc.sync.dma_start(
                                wt[:].rearrange("p (it f) -> p it f", f=OBW),
                                wT_d.ap()[e, nb],
                            )
                            prev_wt = wt
                        for tt in range(TT):
                            pss = []
                            for ob in range(OB):
                                ps = ps_p.tile([128, 512], F32, name=f"ps{ob}")
                                pss.append(ps)
                            if cfg["reorder"]:
                                for it in range(NIT):
                                    xcol = (it * TOK_PER_CORE + e * TPE
                                            + tt * 128)
                                    xs = xt[:, xcol:xcol + 128]
                                    for ob in range(OB):
                                        nc.tensor.matmul(
                                            pss[ob][:], xs,
                                            wt[:, it * OBW + ob * 512:
                                               it * OBW + (ob + 1) * 512],
                                            start=(it == 0),
                                            stop=(it == NIT - 1),
                                        )
                            else:
                                for ob in range(OB):
                                    for it in range(NIT):
                                        xcol = (it * TOK_PER_CORE + e * TPE
                                                + tt * 128)
                                        nc.tensor.matmul(
                                            pss[ob][:], xt[:, xcol:xcol + 128],
                                            wt[:, it * OBW + ob * 512:
                                               it * OBW + (ob + 1) * 512],
                                            start=(it == 0),
                                            stop=(it == NIT - 1),
                                        )
                            if not cfg["probe_skip_evac"]:
                                for ob in range(OB):
                                    gob = nb * OB + ob
                                    nc.vector.tensor_add(
                                        outw[:, tt * OUT_FEATURES + gob * 512:
                                             tt * OUT_FEATURES
                                             + (gob + 1) * 512],
                                        pss[ob][:],
                                        bias_bc[:, gob * 512:(gob + 1) * 512],
                                    )
                    if not cfg["probe_skip_evac"]:
                        y_dma(
                            y_d.ap()[e * TPE:(e + 1) * TPE, :]
                            .rearrange("(tt p) o -> p tt o", p=128),
                            outw[:].rearrange("p (tt o) -> p tt o",
                                              o=OUT_FEATURES),
                        )
    nc.compile()
    return nc


def _get_nc():
    if "nc" not in _CACHE:
        _CACHE["nc"] = _build_nc()
    return _CACHE["nc"]


def _bf16():
    import ml_dtypes
    return ml_dtypes.bfloat16


def _prep_wT(Wc):
    """One core's W [EPC, out, in] -> [EPC, NB, 128, IT, OBW] bf16 with
    per-partition-contiguous (it, f) lines."""
    OBW = DEFAULT_CFG["obw"]
    NB = OUT_FEATURES // OBW
    wt = Wc.astype(_bf16()).transpose(0, 2, 1)  # [EPC, in, out]
    wt = wt.reshape(EPC, IT, 128, NB, OBW).transpose(0, 3, 2, 1, 4)
    return np.ascontiguousarray(wt)


def _prep_xT(xc):
    """One core's x [TOK_PER_CORE, in] -> [128, IT, TOK] bf16."""
    xt = xc.astype(_bf16()).T.reshape(IT, 128, TOK_PER_CORE)
    return np.ascontiguousarray(xt.transpose(1, 0, 2))


def make_in_maps(x, W, b):
    """x [N_CORES*TOK_PER_CORE, in] f32 (pre-padded), W [64, out, in],
    b [64, out] -> per-core input dicts."""
    in_maps = []
    for c in range(N_CORES):
        xc = x[c * TOK_PER_CORE:(c + 1) * TOK_PER_CORE]
        in_maps.append({
            "xT": _prep_xT(xc),
            "wT": _prep_wT(W[c * EPC:(c + 1) * EPC]),
            "b": np.ascontiguousarray(
                b[c * EPC:(c + 1) * EPC].astype(_bf16())),
        })
    return in_maps


def kernel(x, W, b, m_splits):
    from concourse import bass_utils

    x = np.asarray(x, dtype=np.float32)
    W = np.asarray(W, dtype=np.float32)
    b = np.asarray(b, dtype=np.float32)
    splits = [int(c) for c in np.asarray(m_splits)]
    offsets = np.concatenate([[0], np.cumsum(splits)]).astype(np.int64)
    total = int(offsets[-1])

    uniform = all(c == TPE for c in splits)
    if uniform:
        xp = x
    else:
        if max(splits) > TPE:
            # outside the supported regime; fall back to plain numpy
            outs = []
            for i, cnt in enumerate(splits):
                if cnt == 0:
                    continue
                xi = x[offsets[i]:offsets[i] + cnt]
                outs.append(xi @ W[i].T + b[i])
            return np.concatenate(outs, axis=0).astype(np.float32)
        xp = np.zeros((NUM_GEMMS * TPE, IN_FEATURES), dtype=np.float32)
        for i, cnt in enumerate(splits):
            if cnt:
                xp[i * TPE:i * TPE + cnt] = x[offsets[i]:offsets[i] + cnt]

    nc = _get_nc()
    in_maps = make_in_maps(xp, W, b)
    res = bass_utils.run_bass_kernel_spmd(nc, in_maps, core_ids=list(range(N_CORES)))
    yp = np.concatenate(
        [res.results[c]["y"].astype(np.float32) for c in range(N_CORES)], axis=0
    )

    if uniform:
        return yp
    out = np.empty((total, OUT_FEATURES), dtype=np.float32)
    for i, cnt in enumerate(splits):
        if cnt:
            out[offsets[i]:offsets[i] + cnt] = yp[i * TPE:i * TPE + cnt]
    return out


# revision 10
# speedup vs baseline: 1.0734x; 1.0734x over previous
"""Grouped linear (MoE expert GEMM) for Trainium2, 8-core expert-parallel.

Problem: x [16384, 1024] f32, W [64, 4096, 1024] f32, b [64, 4096] f32,
m_splits [64] int64 (host-side counts; 256 each in the reference setup).
y[t] = x[t] @ W[e].T + b[e] for tokens t owned by expert e.

Sharding: expert-parallel — core c owns experts [8c, 8c+8).

Strategy vs the fp32r-split v1 baseline (785us): the correctness gate is
rel_err < 2e-2, so cast x and W to bf16 ON THE HOST. PE runs bf16 at the
same per-row rate as fp32r but needs ONE matmul term instead of the
3-term hi/lo split -> 3x less PE work. Halved W DMA (67MB/core) and bf16
y-out (16.8MB/core) with fp32 PSUM accumulation give rel_err ~3e-3 (6x
under the gate). Measured v2: 174us.

v3 refinements (DGE issue-cost bound, not bandwidth bound):
- Host packs W/x so each partition's block data is one contiguous line
  (16-32KB): 8x fewer DMA descriptors per transfer.
- DMA issue spread across engine queues: W on SP (sync), y on Activation
  (scalar), x + bias on GPSIMD — each hwdge config costs ~0.6-1us of the
  issuing engine's sequencer, so one queue for everything serializes.
- One bias DMA per rep ([8,4096] bf16) + per-expert GPSIMD
  partition_broadcast; one merged y DMA per expert ([256,4096]).
- Matmul loop order it-outer/ob-inner so the stationary x tile is reused
  by consecutive matmuls into different PSUM banks.

Bias is added during PSUM evacuation by DVE (tensor_add with bf16 round).
"""

import numpy as np

NUM_GEMMS = 64
IN_FEATURES = 1024
OUT_FEATURES = 4096
TPE = 256  # tokens per expert slot (padded to this)
N_CORES = 8
EPC = NUM_GEMMS // N_CORES  # experts per core
TOK_PER_CORE = EPC * TPE  # 2048
IT = IN_FEATURES // 128  # 8 contraction tiles
TT = TPE // 128  # 2 token tiles per expert

_CACHE: dict = {}


DEFAULT_CFG = dict(
    obw=1024,        # W DMA block width (outs); OUT_FEATURES % obw == 0
    w_bufs=3,
    x_bufs=2,        # double-buffer x across reps (rep-boundary overlap)
    ps_bufs=4,       # per psum tile name; 2 names x 4 bufs = 8 banks
    reorder=True,    # it-outer/ob-inner matmul order (stationary reuse)
    y_eng="act",     # engine issuing y DMAs: "act" | "sp"
    bias_once=False,  # illegal: partition_broadcast src must be partition 0
    # --- timing probes (wrong results when on; never set in kernel()) ---
    probe_half_w=False,    # reuse previous W tile for odd nb (halves W DMA)
    probe_half_mm=False,   # only 4 of 8 contraction matmuls (halves PE)
    probe_skip_evac=False, # skip DVE evac + bias + y DMA
)


def _build_nc(reps: int = 1, **cfg_over):
    import concourse.bacc as bacc
    import concourse.mybir as mybir
    import concourse.tile as tile

    cfg = {**DEFAULT_CFG, **cfg_over}
    F32 = mybir.dt.float32
    BF16 = mybir.dt.bfloat16
    OBW = cfg["obw"]
    NB = OUT_FEATURES // OBW
    OB = OBW // 512  # psum column tiles per W block
    NIT = 4 if cfg["probe_half_mm"] else IT

    nc = bacc.Bacc(
        "TRN2", target_bir_lowering=False, debug=False, num_devices=N_CORES
    )
    # host-packed layouts (see _prep_*): per-partition-contiguous lines
    xT_d = nc.dram_tensor("xT", [128, IT, TOK_PER_CORE], BF16,
                          kind="ExternalInput")
    wT_d = nc.dram_tensor("wT", [EPC, NB, 128, IT, OBW], BF16,
                          kind="ExternalInput")
    b_d = nc.dram_tensor("b", [EPC, OUT_FEATURES], BF16, kind="ExternalInput")
    y_d = nc.dram_tensor("y", [TOK_PER_CORE, OUT_FEATURES], BF16,
                         kind="ExternalOutput")

    y_dma = nc.scalar.dma_start if cfg["y_eng"] == "act" else nc.sync.dma_start

    with tile.TileContext(nc) as tc:
        with (
            tc.tile_pool(name="xres", bufs=cfg["x_bufs"]) as x_p,
            tc.tile_pool(name="wblk", bufs=cfg["w_bufs"]) as w_p,
            tc.tile_pool(name="bias", bufs=2) as bias_p,
            tc.tile_pool(name="bbc", bufs=2) as bbc_p,
            tc.tile_pool(name="outp", bufs=2) as out_p,
            tc.tile_pool(name="ps", bufs=cfg["ps_bufs"], space="PSUM") as ps_p,
        ):
            for _rep in range(reps):
                # ---- x: one DMA, resident for the whole rep ----
                xt = x_p.tile([128, IT * TOK_PER_CORE], BF16)
                nc.gpsimd.dma_start(
                    xt[:].rearrange("p (it t) -> p it t", t=TOK_PER_CORE),
                    xT_d.ap(),
                )
                ball = None
                if cfg["bias_once"]:
                    ball = bias_p.tile([EPC, OUT_FEATURES], BF16)
                    nc.gpsimd.dma_start(ball[:], b_d.ap())
                for e in range(EPC):
                    if cfg["bias_once"]:
                        bias_src = ball[e:e + 1, :]
                    else:
                        bias_raw = bias_p.tile([1, OUT_FEATURES], BF16)
                        nc.gpsimd.dma_start(bias_raw[:], b_d.ap()[e:e + 1, :])
                        bias_src = bias_raw[:]
                    bias_bc = bbc_p.tile([128, OUT_FEATURES], BF16)
                    nc.gpsimd.partition_broadcast(bias_bc[:], bias_src)

                    outw = out_p.tile([128, TT * OUT_FEATURES], BF16)
                    prev_wt = None
                    for nb in range(NB):
                        if cfg["probe_half_w"] and nb % 2 == 1:
                            wt = prev_wt
                        else:
                            wt = w_p.tile([128, IT * OBW], BF16)
                            nc.sync.dma_start(
                                wt[:].rearrange("p (it f) -> p it f", f=OBW),
                                wT_d.ap()[e, nb],
                            )
                            prev_wt = wt
                        for tt in range(TT):
                            pss = []
                            for ob in range(OB):
                                ps = ps_p.tile([128, 512], F32, name=f"ps{ob}")
                                pss.append(ps)
                            if cfg["reorder"]:
                                for it in range(NIT):
                                    xcol = (it * TOK_PER_CORE + e * TPE
                                            + tt * 128)
                                    xs = xt[:, xcol:xcol + 128]
                                    for ob in range(OB):
                                        nc.tensor.matmul(
                                            pss[ob][:], xs,
                                            wt[:, it * OBW + ob * 512:
                                               it * OBW + (ob + 1) * 512],
                                            start=(it == 0),
                                            stop=(it == NIT - 1),
                                        )
                            else:
                                for ob in range(OB):
                                    for it in range(NIT):
                                        xcol = (it * TOK_PER_CORE + e * TPE
                                                + tt * 128)
                                        nc.tensor.matmul(
                                            pss[ob][:], xt[:, xcol:xcol + 128],
                                            wt[:, it * OBW + ob * 512:
                                               it * OBW + (ob + 1) * 512],
                                            start=(it == 0),
                                            stop=(it == NIT - 1),
                                        )
                            if not cfg["probe_skip_evac"]:
                                for ob in range(OB):
                                    gob = nb * OB + ob
                                    nc.vector.tensor_add(
                                        outw[:, tt * OUT_FEATURES + gob * 512:
                                             tt * OUT_FEATURES
                                             + (gob + 1) * 512],
                                        pss[ob][:],
                                        bias_bc[:, gob * 512:(gob + 1) * 512],
                                    )
                    if not cfg["probe_skip_evac"]:
                        y_dma(
                            y_d.ap()[e * TPE:(e + 1) * TPE, :]
                            .rearrange("(tt p) o -> p tt o", p=128),
                            outw[:].rearrange("p (tt o) -> p tt o",
                                              o=OUT_FEATURES),
                        )
    nc.compile()
    return nc


def _get_nc():
    if "nc" not in _CACHE:
        _CACHE["nc"] = _build_nc()
    return _CACHE["nc"]


def _bf16():
    import ml_dtypes
    return ml_dtypes.bfloat16


def _prep_wT(Wc):
    """One core's W [EPC, out, in] -> [EPC, NB, 128, IT, OBW] bf16 with
    per-partition-contiguous (it, f) lines."""
    OBW = DEFAULT_CFG["obw"]
    NB = OUT_FEATURES // OBW
    wt = Wc.astype(_bf16()).transpose(0, 2, 1)  # [EPC, in, out]
    wt = wt.reshape(EPC, IT, 128, NB, OBW).transpose(0, 3, 2, 1, 4)
    return np.ascontiguousarray(wt)


def _prep_xT(xc):
    """One core's x [TOK_PER_CORE, in] -> [128, IT, TOK] bf16."""
    xt = xc.astype(_bf16()).T.reshape(IT, 128, TOK_PER_CORE)
    return np.ascontiguousarray(xt.transpose(1, 0, 2))


def make_in_maps(x, W, b):
    """x [N_CORES*TOK_PER_CORE, in] f32 (pre-padded), W [64, out, in],
    b [64, out] -> per-core input dicts."""
    in_maps = []
    for c in range(N_CORES):
        xc = x[c * TOK_PER_CORE:(c + 1) * TOK_PER_CORE]
        in_maps.append({
            "xT": _prep_xT(xc),
            "wT": _prep_wT(W[c * EPC:(c + 1) * EPC]),
            "b": np.ascontiguousarray(
                b[c * EPC:(c + 1) * EPC].astype(_bf16())),
        })
    return in_maps


def kernel(x, W, b, m_splits):
    from concourse import bass_utils

    x = np.asarray(x, dtype=np.float32)
    W = np.asarray(W, dtype=np.float32)
    b = np.asarray(b, dtype=np.float32)
    splits = [int(c) for c in np.asarray(m_splits)]
    offsets = np.concatenate([[0], np.cumsum(splits)]).astype(np.int64)
    total = int(offsets[-1])

    uniform = all(c == TPE for c in splits)
    if uniform:
        xp = x
    else:
        if max(splits) > TPE:
            # outside the supported regime; fall back to plain numpy
            outs = []
            for i, cnt in enumerate(splits):
                if cnt == 0:
                    continue
                xi = x[offsets[i]:offsets[i] + cnt]
                outs.append(xi @ W[i].T + b[i])
            return np.concatenate(outs, axis=0).astype(np.float32)
        xp = np.zeros((NUM_GEMMS * TPE, IN_FEATURES), dtype=np.float32)
        for i, cnt in enumerate(splits):
            if cnt:
                xp[i * TPE:i * TPE + cnt] = x[offsets[i]:offsets[i] + cnt]

    nc = _get_nc()
    in_maps = make_in_maps(xp, W, b)
    res = bass_utils.run_bass_kernel_spmd(nc, in_maps, core_ids=list(range(N_CORES)))
    yp = np.concatenate(
        [res.results[c]["y"].astype(np.float32) for c in range(N_CORES)], axis=0
    )

    if uniform:
        return yp
    out = np.empty((total, OUT_FEATURES), dtype=np.float32)
    for i, cnt in enumerate(splits):
        if cnt:
            out[offsets[i]:offsets[i] + cnt] = yp[i * TPE:i * TPE + cnt]
    return out


# revision 16
# speedup vs baseline: 1.2831x; 1.1953x over previous
"""Grouped linear (MoE expert GEMM) for Trainium2, 8-core expert-parallel.

Problem: x [16384, 1024] f32, W [64, 4096, 1024] f32, b [64, 4096] f32,
m_splits [64] int64 (host-side counts; 256 each in the reference setup).
y[t] = x[t] @ W[e].T + b[e] for tokens t owned by expert e.

Sharding: expert-parallel — core c owns experts [8c, 8c+8).

Strategy vs the fp32r-split v1 baseline (785us): the correctness gate is
rel_err < 2e-2, so cast x and W to bf16 ON THE HOST. PE runs bf16 at the
same per-row rate as fp32r but needs ONE matmul term instead of the
3-term hi/lo split -> 3x less PE work. Halved W DMA (67MB/core) and bf16
y-out (16.8MB/core) with fp32 PSUM accumulation give rel_err ~3e-3 (6x
under the gate). Measured v2: 174us.

v3 refinements (DGE issue-cost bound, not bandwidth bound):
- Host packs W/x so each partition's block data is one contiguous line
  (16-32KB): 8x fewer DMA descriptors per transfer.
- DMA issue spread across engine queues: W on SP (sync), y on Activation
  (scalar), x + bias on GPSIMD — each hwdge config costs ~0.6-1us of the
  issuing engine's sequencer, so one queue for everything serializes.
- One bias DMA per rep ([8,4096] bf16) + per-expert GPSIMD
  partition_broadcast; one merged y DMA per expert ([256,4096]).
- Matmul loop order it-outer/ob-inner so the stationary x tile is reused
  by consecutive matmuls into different PSUM banks.

Bias is added during PSUM evacuation by DVE (tensor_add with bf16 round).
"""

import numpy as np

NUM_GEMMS = 64
IN_FEATURES = 1024
OUT_FEATURES = 4096
TPE = 256  # tokens per expert slot (padded to this)
N_CORES = 8
EPC = NUM_GEMMS // N_CORES  # experts per core
TOK_PER_CORE = EPC * TPE  # 2048
IT = IN_FEATURES // 128  # 8 contraction tiles
TT = TPE // 128  # 2 token tiles per expert

_CACHE: dict = {}


DEFAULT_CFG = dict(
    obw=1024,        # W DMA block width (outs); OUT_FEATURES % obw == 0
    w_bufs=4,        # W prefetch depth; 4 measured ~20us/rep over 3
    x_bufs=2,        # double-buffer x across reps (rep-boundary overlap)
    ps_bufs=4,       # per psum tile name; 2 names x 4 bufs = 8 banks
    reorder=True,    # it-outer/ob-inner matmul order (stationary reuse)
    y_eng="act",     # engine issuing y DMAs: "act" | "sp"
    w_split_q=False, # alternate W DMA issue between SP and ACT hw queues
                     # (each queue gets its own DMA-engine allocation);
                     # y moves to the GPSIMD/Pool queue to make room on ACT
    bias_once=False,  # illegal: partition_broadcast src must be partition 0
    # --- timing probes (wrong results when on; never set in kernel()) ---
    probe_half_w=False,    # reuse previous W tile for odd nb (halves W DMA)
    probe_half_mm=False,   # only 4 of 8 contraction matmuls (halves PE)
    probe_skip_evac=False, # skip DVE evac + bias + y DMA
)


def _build_nc(reps: int = 1, **cfg_over):
    import concourse.bacc as bacc
    import concourse.mybir as mybir
    import concourse.tile as tile

    cfg = {**DEFAULT_CFG, **cfg_over}
    F32 = mybir.dt.float32
    BF16 = mybir.dt.bfloat16
    OBW = cfg["obw"]
    NB = OUT_FEATURES // OBW
    OB = OBW // 512  # psum column tiles per W block
    NIT = 4 if cfg["probe_half_mm"] else IT

    nc = bacc.Bacc(
        "TRN2", target_bir_lowering=False, debug=False, num_devices=N_CORES
    )
    # host-packed layouts (see _prep_*): per-partition-contiguous lines
    xT_d = nc.dram_tensor("xT", [128, IT, TOK_PER_CORE], BF16,
                          kind="ExternalInput")
    wT_d = nc.dram_tensor("wT", [EPC, NB, 128, IT, OBW], BF16,
                          kind="ExternalInput")
    b_d = nc.dram_tensor("b", [EPC, OUT_FEATURES], BF16, kind="ExternalInput")
    y_d = nc.dram_tensor("y", [TOK_PER_CORE, OUT_FEATURES], BF16,
                         kind="ExternalOutput")

    if cfg["w_split_q"]:
        y_dma = nc.gpsimd.dma_start
    else:
        y_dma = (nc.scalar.dma_start if cfg["y_eng"] == "act"
                 else nc.sync.dma_start)

    with tile.TileContext(nc) as tc:
        with (
            tc.tile_pool(name="xres", bufs=cfg["x_bufs"]) as x_p,
            tc.tile_pool(name="wblk", bufs=cfg["w_bufs"]) as w_p,
            tc.tile_pool(name="bias", bufs=2) as bias_p,
            tc.tile_pool(name="bbc", bufs=2) as bbc_p,
            tc.tile_pool(name="outp", bufs=2) as out_p,
            tc.tile_pool(name="ps", bufs=cfg["ps_bufs"], space="PSUM") as ps_p,
        ):
            for _rep in range(reps):
                # ---- x: one DMA, resident for the whole rep ----
                xt = x_p.tile([128, IT * TOK_PER_CORE], BF16)
                nc.gpsimd.dma_start(
                    xt[:].rearrange("p (it t) -> p it t", t=TOK_PER_CORE),
                    xT_d.ap(),
                )
                ball = None
                if cfg["bias_once"]:
                    ball = bias_p.tile([EPC, OUT_FEATURES], BF16)
                    nc.gpsimd.dma_start(ball[:], b_d.ap())
                for e in range(EPC):
                    if cfg["bias_once"]:
                        bias_src = ball[e:e + 1, :]
                    else:
                        bias_raw = bias_p.tile([1, OUT_FEATURES], BF16)
                        nc.gpsimd.dma_start(bias_raw[:], b_d.ap()[e:e + 1, :])
                        bias_src = bias_raw[:]
                    bias_bc = bbc_p.tile([128, OUT_FEATURES], BF16)
                    nc.gpsimd.partition_broadcast(bias_bc[:], bias_src)

                    outw = out_p.tile([128, TT * OUT_FEATURES], BF16)
                    prev_wt = None
                    for nb in range(NB):
                        if cfg["probe_half_w"] and nb % 2 == 1:
                            wt = prev_wt
                        else:
                            wt = w_p.tile([128, IT * OBW], BF16)
                            w_eng = (nc.scalar if (cfg["w_split_q"]
                                                   and nb % 2) else nc.sync)
                            w_eng.dma_start(
                                wt[:].rearrange("p (it f) -> p it f", f=OBW),
                                wT_d.ap()[e, nb],
                            )
                            prev_wt = wt
                        for tt in range(TT):
                            pss = []
                            for ob in range(OB):
                                ps = ps_p.tile([128, 512], F32, name=f"ps{ob}")
                                pss.append(ps)
                            if cfg["reorder"]:
                                for it in range(NIT):
                                    xcol = (it * TOK_PER_CORE + e * TPE
                                            + tt * 128)
                                    xs = xt[:, xcol:xcol + 128]
                                    for ob in range(OB):
                                        nc.tensor.matmul(
                                            pss[ob][:], xs,
                                            wt[:, it * OBW + ob * 512:
                                               it * OBW + (ob + 1) * 512],
                                            start=(it == 0),
                                            stop=(it == NIT - 1),
                                        )
                            else:
                                for ob in range(OB):
                                    for it in range(NIT):
                                        xcol = (it * TOK_PER_CORE + e * TPE
                                                + tt * 128)
                                        nc.tensor.matmul(
                                            pss[ob][:], xt[:, xcol:xcol + 128],
                                            wt[:, it * OBW + ob * 512:
                                               it * OBW + (ob + 1) * 512],
                                            start=(it == 0),
                                            stop=(it == NIT - 1),
                                        )
                            if not cfg["probe_skip_evac"]:
                                for ob in range(OB):
                                    gob = nb * OB + ob
                                    nc.vector.tensor_add(
                                        outw[:, tt * OUT_FEATURES + gob * 512:
                                             tt * OUT_FEATURES
                                             + (gob + 1) * 512],
                                        pss[ob][:],
                                        bias_bc[:, gob * 512:(gob + 1) * 512],
                                    )
                    if not cfg["probe_skip_evac"]:
                        y_dma(
                            y_d.ap()[e * TPE:(e + 1) * TPE, :]
                            .rearrange("(tt p) o -> p tt o", p=128),
                            outw[:].rearrange("p (tt o) -> p tt o",
                                              o=OUT_FEATURES),
                        )
    nc.compile()
    return nc


def _get_nc():
    if "nc" not in _CACHE:
        _CACHE["nc"] = _build_nc()
    return _CACHE["nc"]


def _bf16():
    import ml_dtypes
    return ml_dtypes.bfloat16


def _prep_wT(Wc):
    """One core's W [EPC, out, in] -> [EPC, NB, 128, IT, OBW] bf16 with
    per-partition-contiguous (it, f) lines."""
    OBW = DEFAULT_CFG["obw"]
    NB = OUT_FEATURES // OBW
    wt = Wc.astype(_bf16()).transpose(0, 2, 1)  # [EPC, in, out]
    wt = wt.reshape(EPC, IT, 128, NB, OBW).transpose(0, 3, 2, 1, 4)
    return np.ascontiguousarray(wt)


def _prep_xT(xc):
    """One core's x [TOK_PER_CORE, in] -> [128, IT, TOK] bf16."""
    xt = xc.astype(_bf16()).T.reshape(IT, 128, TOK_PER_CORE)
    return np.ascontiguousarray(xt.transpose(1, 0, 2))


def make_in_maps(x, W, b):
    """x [N_CORES*TOK_PER_CORE, in] f32 (pre-padded), W [64, out, in],
    b [64, out] -> per-core input dicts."""
    in_maps = []
    for c in range(N_CORES):
        xc = x[c * TOK_PER_CORE:(c + 1) * TOK_PER_CORE]
        in_maps.append({
            "xT": _prep_xT(xc),
            "wT": _prep_wT(W[c * EPC:(c + 1) * EPC]),
            "b": np.ascontiguousarray(
                b[c * EPC:(c + 1) * EPC].astype(_bf16())),
        })
    return in_maps


def kernel(x, W, b, m_splits):
    from concourse import bass_utils

    x = np.asarray(x, dtype=np.float32)
    W = np.asarray(W, dtype=np.float32)
    b = np.asarray(b, dtype=np.float32)
    splits = [int(c) for c in np.asarray(m_splits)]
    offsets = np.concatenate([[0], np.cumsum(splits)]).astype(np.int64)
    total = int(offsets[-1])

    uniform = all(c == TPE for c in splits)
    if uniform:
        xp = x
    else:
        if max(splits) > TPE:
            # outside the supported regime; fall back to plain numpy
            outs = []
            for i, cnt in enumerate(splits):
                if cnt == 0:
                    continue
                xi = x[offsets[i]:offsets[i] + cnt]
                outs.append(xi @ W[i].T + b[i])
            return np.concatenate(outs, axis=0).astype(np.float32)
        xp = np.zeros((NUM_GEMMS * TPE, IN_FEATURES), dtype=np.float32)
        for i, cnt in enumerate(splits):
            if cnt:
                xp[i * TPE:i * TPE + cnt] = x[offsets[i]:offsets[i] + cnt]

    nc = _get_nc()
    in_maps = make_in_maps(xp, W, b)
    res = bass_utils.run_bass_kernel_spmd(nc, in_maps, core_ids=list(range(N_CORES)))
    yp = np.concatenate(
        [res.results[c]["y"].astype(np.float32) for c in range(N_CORES)], axis=0
    )

    if uniform:
        return yp
    out = np.empty((total, OUT_FEATURES), dtype=np.float32)
    for i, cnt in enumerate(splits):
        if cnt:
            out[offsets[i]:offsets[i] + cnt] = yp[i * TPE:i * TPE + cnt]
    return out


# revision 19
# speedup vs baseline: 1.2947x; 1.0091x over previous
"""Grouped linear (MoE expert GEMM) for Trainium2, 8-core expert-parallel.

Problem: x [16384, 1024] f32, W [64, 4096, 1024] f32, b [64, 4096] f32,
m_splits [64] int64 (host-side counts; 256 each in the reference setup).
y[t] = x[t] @ W[e].T + b[e] for tokens t owned by expert e.

Sharding: expert-parallel — core c owns experts [8c, 8c+8).

Strategy vs the fp32r-split v1 baseline (785us): the correctness gate is
rel_err < 2e-2, so cast x and W to bf16 ON THE HOST. PE runs bf16 at the
same per-row rate as fp32r but needs ONE matmul term instead of the
3-term hi/lo split -> 3x less PE work. Halved W DMA (67MB/core) and bf16
y-out (16.8MB/core) with fp32 PSUM accumulation give rel_err ~3e-3 (6x
under the gate). Measured v2: 174us.

v3 refinements (DGE issue-cost bound, not bandwidth bound):
- Host packs W/x so each partition's block data is one contiguous line
  (16-32KB): 8x fewer DMA descriptors per transfer.
- DMA issue spread across engine queues: W on SP (sync), y on Activation
  (scalar), x + bias on GPSIMD — each hwdge config costs ~0.6-1us of the
  issuing engine's sequencer, so one queue for everything serializes.
- One bias DMA per rep ([8,4096] bf16) + per-expert GPSIMD
  partition_broadcast; one merged y DMA per expert ([256,4096]).
- Matmul loop order it-outer/ob-inner so the stationary x tile is reused
  by consecutive matmuls into different PSUM banks.

Bias is added during PSUM evacuation by DVE (tensor_add with bf16 round).
"""

import numpy as np

NUM_GEMMS = 64
IN_FEATURES = 1024
OUT_FEATURES = 4096
TPE = 256  # tokens per expert slot (padded to this)
N_CORES = 8
EPC = NUM_GEMMS // N_CORES  # experts per core
TOK_PER_CORE = EPC * TPE  # 2048
IT = IN_FEATURES // 128  # 8 contraction tiles
TT = TPE // 128  # 2 token tiles per expert

_CACHE: dict = {}


DEFAULT_CFG = dict(
    obw=1024,        # W DMA block width (outs); OUT_FEATURES % obw == 0
    w_bufs=5,        # W prefetch depth; 4 ~20us/rep over 3; 5 ties/edges 4
    x_bufs=2,        # double-buffer x across reps (rep-boundary overlap)
    bias_bufs=1,     # [1,4096] bias staging; 1 frees the 8KB w_bufs=5 needs
    ps_bufs=4,       # per psum tile name; 2 names x 4 bufs = 8 banks
    reorder=True,    # it-outer/ob-inner matmul order (stationary reuse)
    y_eng="act",     # engine issuing y DMAs: "act" | "sp"
    w_split_q=False, # alternate W DMA issue between SP and ACT hw queues
                     # (each queue gets its own DMA-engine allocation);
                     # y moves to the GPSIMD/Pool queue to make room on ACT
    bias_once=False,  # illegal: partition_broadcast src must be partition 0
    # --- timing probes (wrong results when on; never set in kernel()) ---
    probe_half_w=False,    # reuse previous W tile for odd nb (halves W DMA)
    probe_half_mm=False,   # only 4 of 8 contraction matmuls (halves PE)
    probe_skip_evac=False, # skip DVE evac + bias + y DMA
)


def _build_nc(reps: int = 1, **cfg_over):
    import concourse.bacc as bacc
    import concourse.mybir as mybir
    import concourse.tile as tile

    cfg = {**DEFAULT_CFG, **cfg_over}
    F32 = mybir.dt.float32
    BF16 = mybir.dt.bfloat16
    OBW = cfg["obw"]
    NB = OUT_FEATURES // OBW
    OB = OBW // 512  # psum column tiles per W block
    NIT = 4 if cfg["probe_half_mm"] else IT

    nc = bacc.Bacc(
        "TRN2", target_bir_lowering=False, debug=False, num_devices=N_CORES
    )
    # host-packed layouts (see _prep_*): per-partition-contiguous lines
    xT_d = nc.dram_tensor("xT", [128, IT, TOK_PER_CORE], BF16,
                          kind="ExternalInput")
    wT_d = nc.dram_tensor("wT", [EPC, NB, 128, IT, OBW], BF16,
                          kind="ExternalInput")
    b_d = nc.dram_tensor("b", [EPC, OUT_FEATURES], BF16, kind="ExternalInput")
    y_d = nc.dram_tensor("y", [TOK_PER_CORE, OUT_FEATURES], BF16,
                         kind="ExternalOutput")

    if cfg["w_split_q"]:
        y_dma = nc.gpsimd.dma_start
    else:
        y_dma = (nc.scalar.dma_start if cfg["y_eng"] == "act"
                 else nc.sync.dma_start)

    with tile.TileContext(nc) as tc:
        with (
            tc.tile_pool(name="xres", bufs=cfg["x_bufs"]) as x_p,
            tc.tile_pool(name="wblk", bufs=cfg["w_bufs"]) as w_p,
            tc.tile_pool(name="bias", bufs=cfg["bias_bufs"]) as bias_p,
            tc.tile_pool(name="bbc", bufs=2) as bbc_p,
            tc.tile_pool(name="outp", bufs=2) as out_p,
            tc.tile_pool(name="ps", bufs=cfg["ps_bufs"], space="PSUM") as ps_p,
        ):
            for _rep in range(reps):
                # ---- x: one DMA, resident for the whole rep ----
                xt = x_p.tile([128, IT * TOK_PER_CORE], BF16)
                nc.gpsimd.dma_start(
                    xt[:].rearrange("p (it t) -> p it t", t=TOK_PER_CORE),
                    xT_d.ap(),
                )
                ball = None
                if cfg["bias_once"]:
                    ball = bias_p.tile([EPC, OUT_FEATURES], BF16)
                    nc.gpsimd.dma_start(ball[:], b_d.ap())
                for e in range(EPC):
                    if cfg["bias_once"]:
                        bias_src = ball[e:e + 1, :]
                    else:
                        bias_raw = bias_p.tile([1, OUT_FEATURES], BF16)
                        nc.gpsimd.dma_start(bias_raw[:], b_d.ap()[e:e + 1, :])
                        bias_src = bias_raw[:]
                    bias_bc = bbc_p.tile([128, OUT_FEATURES], BF16)
                    nc.gpsimd.partition_broadcast(bias_bc[:], bias_src)

                    outw = out_p.tile([128, TT * OUT_FEATURES], BF16)
                    prev_wt = None
                    for nb in range(NB):
                        if cfg["probe_half_w"] and nb % 2 == 1:
                            wt = prev_wt
                        else:
                            wt = w_p.tile([128, IT * OBW], BF16)
                            w_eng = (nc.scalar if (cfg["w_split_q"]
                                                   and nb % 2) else nc.sync)
                            w_eng.dma_start(
                                wt[:].rearrange("p (it f) -> p it f", f=OBW),
                                wT_d.ap()[e, nb],
                            )
                            prev_wt = wt
                        for tt in range(TT):
                            pss = []
                            for ob in range(OB):
                                ps = ps_p.tile([128, 512], F32, name=f"ps{ob}")
                                pss.append(ps)
                            if cfg["reorder"]:
                                for it in range(NIT):
                                    xcol = (it * TOK_PER_CORE + e * TPE
                                            + tt * 128)
                                    xs = xt[:, xcol:xcol + 128]
                                    for ob in range(OB):
                                        nc.tensor.matmul(
                                            pss[ob][:], xs,
                                            wt[:, it * OBW + ob * 512:
                                               it * OBW + (ob + 1) * 512],
                                            start=(it == 0),
                                            stop=(it == NIT - 1),
                                        )
                            else:
                                for ob in range(OB):
                                    for it in range(NIT):
                                        xcol = (it * TOK_PER_CORE + e * TPE
                                                + tt * 128)
                                        nc.tensor.matmul(
                                            pss[ob][:], xt[:, xcol:xcol + 128],
                                            wt[:, it * OBW + ob * 512:
                                               it * OBW + (ob + 1) * 512],
                                            start=(it == 0),
                                            stop=(it == NIT - 1),
                                        )
                            if not cfg["probe_skip_evac"]:
                                for ob in range(OB):
                                    gob = nb * OB + ob
                                    nc.vector.tensor_add(
                                        outw[:, tt * OUT_FEATURES + gob * 512:
                                             tt * OUT_FEATURES
                                             + (gob + 1) * 512],
                                        pss[ob][:],
                                        bias_bc[:, gob * 512:(gob + 1) * 512],
                                    )
                    if not cfg["probe_skip_evac"]:
                        y_dma(
                            y_d.ap()[e * TPE:(e + 1) * TPE, :]
                            .rearrange("(tt p) o -> p tt o", p=128),
                            outw[:].rearrange("p (tt o) -> p tt o",
                                              o=OUT_FEATURES),
                        )
    nc.compile()
    return nc


def _get_nc():
    if "nc" not in _CACHE:
        _CACHE["nc"] = _build_nc()
    return _CACHE["nc"]


def _bf16():
    import ml_dtypes
    return ml_dtypes.bfloat16


def _prep_wT(Wc):
    """One core's W [EPC, out, in] -> [EPC, NB, 128, IT, OBW] bf16 with
    per-partition-contiguous (it, f) lines."""
    OBW = DEFAULT_CFG["obw"]
    NB = OUT_FEATURES // OBW
    wt = Wc.astype(_bf16()).transpose(0, 2, 1)  # [EPC, in, out]
    wt = wt.reshape(EPC, IT, 128, NB, OBW).transpose(0, 3, 2, 1, 4)
    return np.ascontiguousarray(wt)


def _prep_xT(xc):
    """One core's x [TOK_PER_CORE, in] -> [128, IT, TOK] bf16."""
    xt = xc.astype(_bf16()).T.reshape(IT, 128, TOK_PER_CORE)
    return np.ascontiguousarray(xt.transpose(1, 0, 2))


def make_in_maps(x, W, b):
    """x [N_CORES*TOK_PER_CORE, in] f32 (pre-padded), W [64, out, in],
    b [64, out] -> per-core input dicts."""
    in_maps = []
    for c in range(N_CORES):
        xc = x[c * TOK_PER_CORE:(c + 1) * TOK_PER_CORE]
        in_maps.append({
            "xT": _prep_xT(xc),
            "wT": _prep_wT(W[c * EPC:(c + 1) * EPC]),
            "b": np.ascontiguousarray(
                b[c * EPC:(c + 1) * EPC].astype(_bf16())),
        })
    return in_maps


def kernel(x, W, b, m_splits):
    from concourse import bass_utils

    x = np.asarray(x, dtype=np.float32)
    W = np.asarray(W, dtype=np.float32)
    b = np.asarray(b, dtype=np.float32)
    splits = [int(c) for c in np.asarray(m_splits)]
    offsets = np.concatenate([[0], np.cumsum(splits)]).astype(np.int64)
    total = int(offsets[-1])

    uniform = all(c == TPE for c in splits)
    if uniform:
        xp = x
    else:
        if max(splits) > TPE:
            # outside the supported regime; fall back to plain numpy
            outs = []
            for i, cnt in enumerate(splits):
                if cnt == 0:
                    continue
                xi = x[offsets[i]:offsets[i] + cnt]
                outs.append(xi @ W[i].T + b[i])
            return np.concatenate(outs, axis=0).astype(np.float32)
        xp = np.zeros((NUM_GEMMS * TPE, IN_FEATURES), dtype=np.float32)
        for i, cnt in enumerate(splits):
            if cnt:
                xp[i * TPE:i * TPE + cnt] = x[offsets[i]:offsets[i] + cnt]

    nc = _get_nc()
    in_maps = make_in_maps(xp, W, b)
    res = bass_utils.run_bass_kernel_spmd(nc, in_maps, core_ids=list(range(N_CORES)))
    yp = np.concatenate(
        [res.results[c]["y"].astype(np.float32) for c in range(N_CORES)], axis=0
    )

    if uniform:
        return yp
    out = np.empty((total, OUT_FEATURES), dtype=np.float32)
    for i, cnt in enumerate(splits):
        if cnt:
            out[offsets[i]:offsets[i] + cnt] = yp[i * TPE:i * TPE + cnt]
    return out
